# revision 1
# baseline (speedup 1.0000x reference)
"""CrossAttentionFusion forward on 8 Trainium2 NeuronCores (pure data parallel).

Math folded on host (seq-len-1 MHA == two chained linears):
  d_att = micro @ A_dm + c_dm,  A_dm = Wv_dm.T @ Wout_dm.T
  m_att = drug  @ A_md + c_md
  u = drug + d_att ; w = micro + m_att
  xu = (u - mu)/sd ; xw likewise        (LN affine folded into W1)
  h1 = gelu([xu, xw] @ W1f + b1f),  W1f = (ffn_w1 * g_cat).T
  h2 = h1 @ W2f + b2,               W2f = ffn_w2.T
  out = ((h2 - mu)/sd) * g_out + b_out

Device layout: activations feature-major [feat(partition), batch(free)];
batch sharded across 8 cores, tiles of NB=512 columns. LN stats via
ones-matmuls, per-column mean/rstd broadcast across partitions via K=1
matmuls. All matmuls run bf16 with fp32 PSUM accumulation.
"""

import sys

if "/opt/trn_rl_repo" not in sys.path:
    sys.path.insert(0, "/opt/trn_rl_repo")

from contextlib import ExitStack

import ml_dtypes
import numpy as np

import concourse.bass as bass  # noqa: F401  (registers mybir lowering hooks)
import concourse.tile as tile
from concourse import bacc, mybir
from concourse.bass import ts
from concourse.bass_utils import run_bass_kernel_spmd

F32 = mybir.dt.float32
F32R = mybir.dt.float32r
BF16 = mybir.dt.bfloat16
ACT = mybir.ActivationFunctionType

P = 128
D = 384
KD = D // P          # 3
DH = 2 * D           # 768
KH = DH // P         # 6
DF = 4 * D           # 1536
KF = DF // P         # 12
EPS = 1e-5
N_CORES = 8
B_FULL = 65536
BC = B_FULL // N_CORES   # 8192 rows per core
NB = 512                 # batch columns per on-chip tile

_NC_CACHE = {}
LAST_RESULTS = None      # BassKernelResults of the most recent kernel() call


def _build_nc(bc, nb, flags):
    use_c_dm, use_c_md, use_b1, use_b2, use_affine = flags
    nt = bc // nb
    nc = bacc.Bacc("TRN2", target_bir_lowering=False, debug=False,
                   num_devices=N_CORES)

    xd_d = nc.dram_tensor("xd", [D, bc], BF16, kind="ExternalInput")
    xm_d = nc.dram_tensor("xm", [D, bc], BF16, kind="ExternalInput")
    a_dm_d = nc.dram_tensor("a_dm", [D, D], BF16, kind="ExternalInput")
    a_md_d = nc.dram_tensor("a_md", [D, D], BF16, kind="ExternalInput")
    w1_d = nc.dram_tensor("w1", [DH, DF], BF16, kind="ExternalInput")
    w2_d = nc.dram_tensor("w2", [DF, D], BF16, kind="ExternalInput")
    c_dm_d = nc.dram_tensor("c_dm", [D], F32, kind="ExternalInput") if use_c_dm else None
    c_md_d = nc.dram_tensor("c_md", [D], F32, kind="ExternalInput") if use_c_md else None
    b1_d = nc.dram_tensor("b1", [DF], F32, kind="ExternalInput") if use_b1 else None
    b2_d = nc.dram_tensor("b2", [D], F32, kind="ExternalInput") if use_b2 else None
    g_o_d = nc.dram_tensor("g_o", [D], F32, kind="ExternalInput") if use_affine else None
    b_o_d = nc.dram_tensor("b_o", [D], F32, kind="ExternalInput") if use_affine else None
    o_d = nc.dram_tensor("o", [D, bc], F32, kind="ExternalOutput")

    xd_r = xd_d.ap().rearrange("(k p) n -> p k n", p=P)
    xm_r = xm_d.ap().rearrange("(k p) n -> p k n", p=P)
    o_r = o_d.ap().rearrange("(k p) n -> p k n", p=P)

    with tile.TileContext(nc) as tc, ExitStack() as ctx:
        wp = ctx.enter_context(tc.tile_pool(name="wts", bufs=1))
        xp = ctx.enter_context(tc.tile_pool(name="x", bufs=3))
        up = ctx.enter_context(tc.tile_pool(name="u", bufs=3))
        sqp = ctx.enter_context(tc.tile_pool(name="sq", bufs=3))
        xhp = ctx.enter_context(tc.tile_pool(name="xh", bufs=3))
        h1p = ctx.enter_context(tc.tile_pool(name="h1", bufs=2))
        h2p = ctx.enter_context(tc.tile_pool(name="h2", bufs=2))
        op_ = ctx.enter_context(tc.tile_pool(name="o", bufs=2))
        stp = ctx.enter_context(tc.tile_pool(name="st", bufs=3))
        pmm = ctx.enter_context(tc.tile_pool(name="pmm", bufs=4, space="PSUM"))
        pst = ctx.enter_context(tc.tile_pool(name="pst", bufs=2, space="PSUM"))
        pbc = ctx.enter_context(tc.tile_pool(name="pbc", bufs=2, space="PSUM"))

        a_dm_sb = wp.tile([P, KD, D], BF16)
        nc.gpsimd.dma_start(a_dm_sb[:], a_dm_d.ap().rearrange("(k p) m -> p k m", p=P))
        a_md_sb = wp.tile([P, KD, D], BF16)
        nc.gpsimd.dma_start(a_md_sb[:], a_md_d.ap().rearrange("(k p) m -> p k m", p=P))
        w1_sb = wp.tile([P, KH, DF], BF16)
        nc.gpsimd.dma_start(w1_sb[:], w1_d.ap().rearrange("(k p) m -> p k m", p=P))
        w2_sb = wp.tile([P, KF, D], BF16)
        nc.gpsimd.dma_start(w2_sb[:], w2_d.ap().rearrange("(k p) m -> p k m", p=P))

        ones_mean = wp.tile([P, 1], BF16)
        nc.vector.memset(ones_mean[:], 1.0 / D)
        ones16 = wp.tile([P, 1], BF16)
        nc.vector.memset(ones16[:], 1.0)
        ones_bc = wp.tile([1, P], BF16)
        nc.vector.memset(ones_bc[:], 1.0)
        eps_sb = wp.tile([1, 1], F32)
        nc.vector.memset(eps_sb[:], EPS)

        def vec_const(dram, nk, tag):
            t = wp.tile([P, nk], F32, tag=tag)
            nc.gpsimd.dma_start(t[:], dram.ap().rearrange("(k p) -> p k", p=P))
            return t

        c_dm_sb = vec_const(c_dm_d, KD, "c_dm") if use_c_dm else None
        c_md_sb = vec_const(c_md_d, KD, "c_md") if use_c_md else None
        b1_sb = vec_const(b1_d, KF, "b1") if use_b1 else None
        b2_sb = vec_const(b2_d, KD, "b2") if use_b2 else None
        g_o_sb = vec_const(g_o_d, KD, "g_o") if use_affine else None
        b_o_sb = vec_const(b_o_d, KD, "b_o") if use_affine else None

        def ln_stats(u, nk):
            """u: [P, nk, nb] fp32 SBUF -> (MU_ps, INV_ps) [P, nb] PSUM."""
            s_ps = pst.tile([1, NB], F32, tag="st")
            for k in range(nk):
                nc.tensor.matmul(s_ps[:], ones_mean[:], u[:, k, :],
                                 start=(k == 0), stop=(k == nk - 1))
            sq = sqp.tile([P, nk, NB], BF16, tag="sq")
            for k in range(nk):
                nc.scalar.activation(sq[:, k, :], u[:, k, :], ACT.Square,
                                     scale=float(1.0 / np.sqrt(D)))
            s2_ps = pst.tile([1, NB], F32, tag="st")
            for k in range(nk):
                nc.tensor.matmul(s2_ps[:], ones16[:], sq[:, k, :],
                                 start=(k == 0), stop=(k == nk - 1))
            mu = stp.tile([1, NB], BF16, tag="mu")
            nc.scalar.activation(mu[:], s_ps[:], ACT.Copy)
            mu2 = stp.tile([1, NB], F32, tag="mu2")
            nc.vector.tensor_mul(mu2[:], mu[:], mu[:])
            var = stp.tile([1, NB], F32, tag="var")
            nc.vector.tensor_sub(var[:], s2_ps[:], mu2[:])
            sd = stp.tile([1, NB], F32, tag="sd")
            nc.scalar.activation(sd[:], var[:], ACT.Sqrt, bias=eps_sb[:])
            inv = stp.tile([1, NB], F32, tag="inv")
            nc.vector.reciprocal_approx_fast(inv[:], sd[:])
            inv16 = stp.tile([1, NB], BF16, tag="inv16")
            nc.vector.tensor_copy(inv16[:], inv[:])
            mu_ps = pbc.tile([P, NB], F32, tag="bc")
            nc.tensor.matmul(mu_ps[:], ones_bc[:], mu[:],
                             start=True, stop=True)
            inv_ps = pbc.tile([P, NB], F32, tag="bc")
            nc.tensor.matmul(inv_ps[:], ones_bc[:], inv16[:],
                             start=True, stop=True)
            return mu_ps, inv_ps

        def out_ln_store(h2, osl):
            mu_o, inv_o = ln_stats(h2, KD)
            o = op_.tile([P, KD, NB], F32, tag="o")
            for k in range(KD):
                nc.vector.tensor_sub(h2[:, k, :], h2[:, k, :], mu_o[:])
                nc.vector.tensor_mul(o[:, k, :], h2[:, k, :], inv_o[:])
                if use_affine:
                    nc.vector.tensor_scalar(o[:, k, :], o[:, k, :],
                                            g_o_sb[:, k:k + 1],
                                            b_o_sb[:, k:k + 1],
                                            mybir.AluOpType.mult,
                                            mybir.AluOpType.add)
            nc.sync.dma_start(o_r[:, :, osl], o[:])

        prev = None
        for it in range(nt):
            sl = slice(it * NB, (it + 1) * NB)
            xd = xp.tile([P, KD, NB], BF16, tag="xd")
            nc.sync.dma_start(xd[:], xd_r[:, :, sl])
            xm = xp.tile([P, KD, NB], BF16, tag="xm")
            nc.sync.dma_start(xm[:], xm_r[:, :, sl])

            def attn(a_sb, rhs, res, c_sb, tag):
                v = up.tile([P, KD, NB], BF16, tag=tag)
                for m in range(KD):
                    ps = pmm.tile([P, NB], F32, tag="mm")
                    for k in range(KD):
                        nc.tensor.matmul(ps[:],
                                         a_sb[:, k, ts(m, P)],
                                         rhs[:, k, :],
                                         start=(k == 0), stop=(k == KD - 1))
                    nc.vector.tensor_add(v[:, m, :], ps[:], res[:, m, :])
                    if c_sb is not None:
                        nc.vector.tensor_scalar_add(v[:, m, :], v[:, m, :],
                                                    c_sb[:, m:m + 1])
                return v

            u = attn(a_dm_sb, xm, xd, c_dm_sb, "u")
            w = attn(a_md_sb, xd, xm, c_md_sb, "w")

            if prev is not None:
                out_ln_store(*prev)
                prev = None

            mu_u, inv_u = ln_stats(u, KD)
            xhu = xhp.tile([P, KD, NB], BF16, tag="xhu")
            for k in range(KD):
                nc.vector.tensor_sub(u[:, k, :], u[:, k, :], mu_u[:])
                nc.vector.tensor_mul(xhu[:, k, :], u[:, k, :], inv_u[:])
            mu_w, inv_w = ln_stats(w, KD)
            xhw = xhp.tile([P, KD, NB], BF16, tag="xhw")
            for k in range(KD):
                nc.vector.tensor_sub(w[:, k, :], w[:, k, :], mu_w[:])
                nc.vector.tensor_mul(xhw[:, k, :], w[:, k, :], inv_w[:])

            h1 = h1p.tile([P, KF, NB], BF16, tag="h1")
            for m in range(KF):
                ps = pmm.tile([P, NB], F32, tag="mm")
                for k in range(KH):
                    rhs = xhu[:, k, :] if k < KD else xhw[:, k - KD, :]
                    nc.tensor.matmul(ps[:], w1_sb[:, k, ts(m, P)], rhs,
                                     start=(k == 0), stop=(k == KH - 1))
                if use_b1:
                    nc.scalar.activation(h1[:, m, :], ps[:], ACT.Gelu,
                                         bias=b1_sb[:, m:m + 1])
                else:
                    nc.scalar.activation(h1[:, m, :], ps[:], ACT.Gelu)

            h2 = h2p.tile([P, KD, NB], BF16, tag="h2")
            for m in range(KD):
                ps = pmm.tile([P, NB], F32, tag="mm")
                for k in range(KF):
                    nc.tensor.matmul(ps[:], w2_sb[:, k, ts(m, P)], h1[:, k, :],
                                     start=(k == 0), stop=(k == KF - 1))
                if use_b2:
                    nc.scalar.activation(h2[:, m, :], ps[:], ACT.Identity,
                                         bias=b2_sb[:, m:m + 1])
                else:
                    nc.scalar.activation(h2[:, m, :], ps[:], ACT.Copy)

            prev = (h2, sl)

        out_ln_store(*prev)

    nc.compile()
    return nc


def kernel(**inputs) -> np.ndarray:
    global LAST_RESULTS
    f = lambda k: np.asarray(inputs[k], np.float32)

    drug = f("drug_emb")
    micro = f("micro_emb")
    b = drug.shape[0]
    bc = b // N_CORES
    assert b % (N_CORES * NB) == 0

    # ---- host-side weight folding ----
    wv_dm, bv_dm = f("dm_in_w")[2 * D:], f("dm_in_b")[2 * D:]
    wv_md, bv_md = f("md_in_w")[2 * D:], f("md_in_b")[2 * D:]
    a_dm = np.ascontiguousarray(wv_dm.T @ f("dm_out_w").T).astype(ml_dtypes.bfloat16)
    c_dm = bv_dm @ f("dm_out_w").T + f("dm_out_b")
    a_md = np.ascontiguousarray(wv_md.T @ f("md_out_w").T).astype(ml_dtypes.bfloat16)
    c_md = bv_md @ f("md_out_w").T + f("md_out_b")
    g_cat = np.concatenate([f("norm_d_g"), f("norm_m_g")])
    b_cat = np.concatenate([f("norm_d_b"), f("norm_m_b")])
    w1f = np.ascontiguousarray((f("ffn_w1") * g_cat[None, :]).T).astype(ml_dtypes.bfloat16)
    b1f = f("ffn_b1") + b_cat @ f("ffn_w1").T
    w2f = np.ascontiguousarray(f("ffn_w2").T).astype(ml_dtypes.bfloat16)
    b2 = f("ffn_b2")
    g_o, b_o = f("norm_out_g"), f("norm_out_b")

    flags = (bool(np.any(c_dm)), bool(np.any(c_md)), bool(np.any(b1f)),
             bool(np.any(b2)), bool(np.any(g_o != 1.0) or np.any(b_o)))

    key = (bc, NB, flags)
    if key not in _NC_CACHE:
        _NC_CACHE[key] = _build_nc(bc, NB, flags)
    nc = _NC_CACHE[key]

    in_maps = []
    for c in range(N_CORES):
        sl = slice(c * bc, (c + 1) * bc)
        m = {
            "xd": np.ascontiguousarray(drug[sl].T).astype(ml_dtypes.bfloat16),
            "xm": np.ascontiguousarray(micro[sl].T).astype(ml_dtypes.bfloat16),
            "a_dm": a_dm, "a_md": a_md, "w1": w1f, "w2": w2f,
        }
        if flags[0]:
            m["c_dm"] = c_dm
        if flags[1]:
            m["c_md"] = c_md
        if flags[2]:
            m["b1"] = b1f
        if flags[3]:
            m["b2"] = b2
        if flags[4]:
            m["g_o"] = g_o
            m["b_o"] = b_o
        in_maps.append(m)

    res = run_bass_kernel_spmd(nc, in_maps, list(range(N_CORES)))
    LAST_RESULTS = res

    out = np.empty((b, D), np.float32)
    for c in range(N_CORES):
        out[c * bc:(c + 1) * bc] = res.results[c]["o"].T
    return out



# revision 3
# speedup vs baseline: 1.1535x; 1.1535x over previous
"""CrossAttentionFusion forward on 8 Trainium2 NeuronCores (pure data parallel).

Math folded on host (seq-len-1 MHA == two chained linears):
  d_att = micro @ A_dm + c_dm,  A_dm = Wv_dm.T @ Wout_dm.T
  m_att = drug  @ A_md + c_md
  u = drug + d_att ; w = micro + m_att
  xu = (u - mu)/sd ; xw likewise        (LN affine folded into W1)
  h1 = gelu([xu, xw] @ W1f + b1f),  W1f = (ffn_w1 * g_cat).T
  h2 = h1 @ W2f + b2,               W2f = ffn_w2.T
  out = ((h2 - mu)/sd) * g_out + b_out

Device layout: feature-major [feat(partition), batch(free)], batch sharded
across 8 cores, NB=256 batch columns per tile (32 tiles/core).

Software pipeline per iteration t (PE program order):
  attn(t) -> out-LN(t-2) -> FFN1(t-1) -> LN-stats(t) -> FFN2(t-1)
so the cross-engine LayerNorm stat chains (PE stats -> DVE smalls ->
GpSimd partition-broadcast -> DVE normalize) always have ~8us of
independent PE work in front of their consumers.  LN mean/rstd are
broadcast across partitions by GpSimd (partition_broadcast), not by PE
matmuls, freeing PE cycles and PSUM banks.
"""

import sys

if "/opt/trn_rl_repo" not in sys.path:
    sys.path.insert(0, "/opt/trn_rl_repo")

from contextlib import ExitStack

import ml_dtypes
import numpy as np

import concourse.bass as bass  # noqa: F401  (registers mybir lowering hooks)
import concourse.tile as tile
from concourse import bacc, mybir
from concourse.bass import ts, broadcast_tensor_aps
from concourse.bass_utils import run_bass_kernel_spmd

F32 = mybir.dt.float32
BF16 = mybir.dt.bfloat16
ACT = mybir.ActivationFunctionType

P = 128
D = 384
KD = D // P          # 3
DH = 2 * D           # 768
KH = DH // P         # 6
DF = 4 * D           # 1536
KF = DF // P         # 12
EPS = 1e-5
N_CORES = 8
B_FULL = 65536
BC = B_FULL // N_CORES   # 8192 rows per core
NB = 256                 # batch columns per on-chip tile
NT = BC // NB            # 32 tiles per core

_NC_CACHE = {}
LAST_RESULTS = None      # BassKernelResults of the most recent kernel() call


def _build_nc(bc, flags):
    use_c_dm, use_c_md, use_b1, use_b2, use_affine = flags
    nt = bc // NB
    nc = bacc.Bacc("TRN2", target_bir_lowering=False, debug=False,
                   num_devices=N_CORES)

    xd_d = nc.dram_tensor("xd", [D, bc], BF16, kind="ExternalInput")
    xm_d = nc.dram_tensor("xm", [D, bc], BF16, kind="ExternalInput")
    a_dm_d = nc.dram_tensor("a_dm", [D, D], BF16, kind="ExternalInput")
    a_md_d = nc.dram_tensor("a_md", [D, D], BF16, kind="ExternalInput")
    w1_d = nc.dram_tensor("w1", [DH, DF], BF16, kind="ExternalInput")
    w2_d = nc.dram_tensor("w2", [DF, D], BF16, kind="ExternalInput")
    c_dm_d = nc.dram_tensor("c_dm", [D], F32, kind="ExternalInput") if use_c_dm else None
    c_md_d = nc.dram_tensor("c_md", [D], F32, kind="ExternalInput") if use_c_md else None
    b1_d = nc.dram_tensor("b1", [DF], F32, kind="ExternalInput") if use_b1 else None
    b2_d = nc.dram_tensor("b2", [D], F32, kind="ExternalInput") if use_b2 else None
    g_o_d = nc.dram_tensor("g_o", [D], F32, kind="ExternalInput") if use_affine else None
    b_o_d = nc.dram_tensor("b_o", [D], F32, kind="ExternalInput") if use_affine else None
    o_d = nc.dram_tensor("o", [D, bc], F32, kind="ExternalOutput")

    xd_r = xd_d.ap().rearrange("(k p) n -> p k n", p=P)
    xm_r = xm_d.ap().rearrange("(k p) n -> p k n", p=P)
    o_r = o_d.ap().rearrange("(k p) n -> p k n", p=P)

    with tile.TileContext(nc) as tc, ExitStack() as ctx:
        wp = ctx.enter_context(tc.tile_pool(name="wts", bufs=1))
        xp = ctx.enter_context(tc.tile_pool(name="x", bufs=3))
        up = ctx.enter_context(tc.tile_pool(name="u", bufs=2))
        sqp = ctx.enter_context(tc.tile_pool(name="sq", bufs=2))
        xhp = ctx.enter_context(tc.tile_pool(name="xh", bufs=2))
        h1p = ctx.enter_context(tc.tile_pool(name="h1", bufs=2))
        h2p = ctx.enter_context(tc.tile_pool(name="h2", bufs=3))
        op_ = ctx.enter_context(tc.tile_pool(name="o", bufs=2))
        stp = ctx.enter_context(tc.tile_pool(name="st", bufs=4))
        bcp = ctx.enter_context(tc.tile_pool(name="bc", bufs=2))
        # PSUM: pbig 2 bufs x 2 banks + pf1 2 bufs x 1 bank + pst 2 banks = 8
        pbig = ctx.enter_context(tc.tile_pool(name="pbig", bufs=2, space="PSUM"))
        pf1 = ctx.enter_context(tc.tile_pool(name="pf1", bufs=2, space="PSUM"))
        pst = ctx.enter_context(tc.tile_pool(name="pst", bufs=1, space="PSUM"))

        a_dm_sb = wp.tile([P, KD, D], BF16)
        nc.gpsimd.dma_start(a_dm_sb[:], a_dm_d.ap().rearrange("(k p) m -> p k m", p=P))
        a_md_sb = wp.tile([P, KD, D], BF16)
        nc.gpsimd.dma_start(a_md_sb[:], a_md_d.ap().rearrange("(k p) m -> p k m", p=P))
        w1_sb = wp.tile([P, KH, DF], BF16)
        nc.gpsimd.dma_start(w1_sb[:], w1_d.ap().rearrange("(k p) m -> p k m", p=P))
        w2_sb = wp.tile([P, KF, D], BF16)
        nc.gpsimd.dma_start(w2_sb[:], w2_d.ap().rearrange("(k p) m -> p k m", p=P))

        ones_mean = wp.tile([P, 1], BF16)
        nc.vector.memset(ones_mean[:], 1.0 / D)
        ones16 = wp.tile([P, 1], BF16)
        nc.vector.memset(ones16[:], 1.0)
        eps_sb = wp.tile([1, 1], F32)
        nc.vector.memset(eps_sb[:], EPS)

        def vec_const(dram, nk, tag):
            t = wp.tile([P, nk], F32, tag=tag)
            nc.gpsimd.dma_start(t[:], dram.ap().rearrange("(k p) -> p k", p=P))
            return t

        c_dm_sb = vec_const(c_dm_d, KD, "c_dm") if use_c_dm else None
        c_md_sb = vec_const(c_md_d, KD, "c_md") if use_c_md else None
        b1_sb = vec_const(b1_d, KF, "b1") if use_b1 else None
        b2_sb = vec_const(b2_d, KD, "b2") if use_b2 else None
        g_o_sb = vec_const(g_o_d, KD, "g_o") if use_affine else None
        b_o_sb = vec_const(b_o_d, KD, "b_o") if use_affine else None

        # Persistent stats PSUM: slots 0..3 = {sum_u, ssq_u, sum_w, ssq_w}
        # (also reused as {sum_o, ssq_o} in slots 0..1 for the out-LN, which
        # runs at a disjoint point of the iteration).
        stat_ps = pst.tile([1, 4, NB], F32)

        def bsub(out_ap, a_ap, b_ap):
            a2, b2_ = broadcast_tensor_aps(a_ap, b_ap)
            o2, _ = broadcast_tensor_aps(out_ap, b_ap)
            nc.vector.tensor_sub(o2, a2, b2_)

        def bmul(out_ap, a_ap, b_ap):
            a2, b2_ = broadcast_tensor_aps(a_ap, b_ap)
            o2, _ = broadcast_tensor_aps(out_ap, b_ap)
            nc.vector.tensor_mul(o2, a2, b2_)

        def ln_smalls(sum_sl, ssq_sl, tag):
            """[1,NB] psum rows -> (mu16, inv16) bf16 SBUF on partition 0."""
            mu16 = stp.tile([1, NB], BF16, tag="mu" + tag)
            nc.vector.tensor_copy(mu16[:], sum_sl)
            mu2 = stp.tile([1, NB], F32, tag="mu2")
            nc.vector.tensor_mul(mu2[:], mu16[:], mu16[:])
            var = stp.tile([1, NB], F32, tag="var")
            nc.vector.tensor_sub(var[:], ssq_sl, mu2[:])
            sd = stp.tile([1, NB], F32, tag="sd")
            nc.scalar.activation(sd[:], var[:], ACT.Sqrt, bias=eps_sb[:])
            inv = stp.tile([1, NB], F32, tag="inv")
            nc.vector.reciprocal_approx_fast(inv[:], sd[:])
            inv16 = stp.tile([1, NB], BF16, tag="inv" + tag)
            nc.vector.tensor_copy(inv16[:], inv[:])
            return mu16, inv16

        def bcast(vec16, tag):
            t = bcp.tile([P, 1, NB], BF16, tag=tag)
            nc.gpsimd.partition_broadcast(t[:, 0, :], vec16[:])
            return t

        state = {}

        def emit_attn(t, sl):
            xd = xp.tile([P, KD, NB], BF16, tag="xd")
            nc.sync.dma_start(xd[:], xd_r[:, :, sl])
            xm = xp.tile([P, KD, NB], BF16, tag="xm")
            nc.sync.dma_start(xm[:], xm_r[:, :, sl])

            def attn(a_sb, rhs, res, c_sb, tag):
                ps = pbig.tile([P, KD, NB], F32, tag="big")
                for m in range(KD):
                    for k in range(KD):
                        nc.tensor.matmul(ps[:, m, :],
                                         a_sb[:, k, ts(m, P)],
                                         rhs[:, k, :],
                                         start=(k == 0), stop=(k == KD - 1))
                v = up.tile([P, KD, NB], BF16, tag=tag)
                nc.vector.tensor_add(v[:], ps[:], res[:])
                if c_sb is not None:
                    for m in range(KD):
                        nc.vector.tensor_scalar_add(v[:, m, :], v[:, m, :],
                                                    c_sb[:, m:m + 1])
                return v

            u = attn(a_dm_sb, xm, xd, c_dm_sb, "u")
            w = attn(a_md_sb, xd, xm, c_md_sb, "w")
            state[t] = {"u": u, "w": w}

        def emit_uw_stats(t):
            st_ = state[t]
            u, w = st_["u"], st_["w"]
            sq_u = sqp.tile([P, KD, NB], BF16, tag="squ")
            nc.scalar.activation(sq_u[:], u[:], ACT.Square,
                                 scale=float(1.0 / np.sqrt(D)))
            sq_w = sqp.tile([P, KD, NB], BF16, tag="sqw")
            nc.scalar.activation(sq_w[:], w[:], ACT.Square,
                                 scale=float(1.0 / np.sqrt(D)))
            for k in range(KD):
                nc.tensor.matmul(stat_ps[0:1, 0, :], ones_mean[:], u[:, k, :],
                                 start=(k == 0), stop=(k == KD - 1))
            for k in range(KD):
                nc.tensor.matmul(stat_ps[0:1, 1, :], ones16[:], sq_u[:, k, :],
                                 start=(k == 0), stop=(k == KD - 1))
            for k in range(KD):
                nc.tensor.matmul(stat_ps[0:1, 2, :], ones_mean[:], w[:, k, :],
                                 start=(k == 0), stop=(k == KD - 1))
            for k in range(KD):
                nc.tensor.matmul(stat_ps[0:1, 3, :], ones16[:], sq_w[:, k, :],
                                 start=(k == 0), stop=(k == KD - 1))
            mu_u, inv_u = ln_smalls(stat_ps[0:1, 0, :], stat_ps[0:1, 1, :], "u")
            mu_w, inv_w = ln_smalls(stat_ps[0:1, 2, :], stat_ps[0:1, 3, :], "w")
            mu_bu = bcast(mu_u, "mbu")
            iv_bu = bcast(inv_u, "ibu")
            mu_bw = bcast(mu_w, "mbw")
            iv_bw = bcast(inv_w, "ibw")
            xh = xhp.tile([P, KH, NB], BF16, tag="xh")
            bsub(u[:], u[:], mu_bu[:])
            bmul(xh[:, 0:KD, :], u[:], iv_bu[:])
            bsub(w[:], w[:], mu_bw[:])
            bmul(xh[:, KD:KH, :], w[:], iv_bw[:])
            st_["xh"] = xh

        def emit_ffn1(t):
            st_ = state[t]
            xh = st_["xh"]
            h1 = h1p.tile([P, KF, NB], BF16, tag="h1")
            for g in range(KF // 2):
                ps = pf1.tile([P, 2, NB], F32, tag="f1")
                for mi in range(2):
                    m = g * 2 + mi
                    for k in range(KH):
                        nc.tensor.matmul(ps[:, mi, :],
                                         w1_sb[:, k, ts(m, P)],
                                         xh[:, k, :],
                                         start=(k == 0), stop=(k == KH - 1))
                if use_b1:
                    for mi in range(2):
                        m = g * 2 + mi
                        nc.scalar.activation(h1[:, m, :], ps[:, mi, :],
                                             ACT.Gelu, bias=b1_sb[:, m:m + 1])
                else:
                    nc.scalar.activation(h1[:, ts(g, 2), :], ps[:], ACT.Gelu)
            st_["h1"] = h1

        def emit_ffn2(t):
            st_ = state[t]
            h1 = st_["h1"]
            ps = pbig.tile([P, KD, NB], F32, tag="big")
            for m in range(KD):
                for k in range(KF):
                    nc.tensor.matmul(ps[:, m, :], w2_sb[:, k, ts(m, P)],
                                     h1[:, k, :],
                                     start=(k == 0), stop=(k == KF - 1))
            h2 = h2p.tile([P, KD, NB], BF16, tag="h2")
            if use_b2:
                for m in range(KD):
                    nc.scalar.activation(h2[:, m, :], ps[:, m, :],
                                         ACT.Identity, bias=b2_sb[:, m:m + 1])
            else:
                nc.scalar.activation(h2[:], ps[:], ACT.Copy)
            st_["h2"] = h2

        def emit_outln(t, sl):
            st_ = state.pop(t)
            h2 = st_["h2"]
            sq_o = sqp.tile([P, KD, NB], BF16, tag="sqo")
            nc.scalar.activation(sq_o[:], h2[:], ACT.Square,
                                 scale=float(1.0 / np.sqrt(D)))
            for k in range(KD):
                nc.tensor.matmul(stat_ps[0:1, 0, :], ones_mean[:], h2[:, k, :],
                                 start=(k == 0), stop=(k == KD - 1))
            for k in range(KD):
                nc.tensor.matmul(stat_ps[0:1, 1, :], ones16[:], sq_o[:, k, :],
                                 start=(k == 0), stop=(k == KD - 1))
            mu_o, inv_o = ln_smalls(stat_ps[0:1, 0, :], stat_ps[0:1, 1, :], "o")
            mu_bo = bcast(mu_o, "mbo")
            iv_bo = bcast(inv_o, "ibo")
            o = op_.tile([P, KD, NB], F32, tag="o")
            bsub(o[:], h2[:], mu_bo[:])
            bmul(o[:], o[:], iv_bo[:])
            if use_affine:
                for k in range(KD):
                    nc.vector.tensor_scalar(o[:, k, :], o[:, k, :],
                                            g_o_sb[:, k:k + 1],
                                            b_o_sb[:, k:k + 1],
                                            mybir.AluOpType.mult,
                                            mybir.AluOpType.add)
            nc.sync.dma_start(o_r[:, :, sl], o[:])

        for t in range(nt + 2):
            sl = slice(t * NB, (t + 1) * NB)
            if t < nt:
                emit_attn(t, sl)
            if t >= 2:
                osl = slice((t - 2) * NB, (t - 1) * NB)
                emit_outln(t - 2, osl)
            if 1 <= t <= nt:
                emit_ffn1(t - 1)
            if t < nt:
                emit_uw_stats(t)
            if 1 <= t <= nt:
                emit_ffn2(t - 1)

    nc.compile()
    return nc


def kernel(**inputs) -> np.ndarray:
    global LAST_RESULTS
    f = lambda k: np.asarray(inputs[k], np.float32)

    drug = f("drug_emb")
    micro = f("micro_emb")
    b = drug.shape[0]
    bc = b // N_CORES
    assert b % (N_CORES * NB) == 0

    # ---- host-side weight folding ----
    wv_dm, bv_dm = f("dm_in_w")[2 * D:], f("dm_in_b")[2 * D:]
    wv_md, bv_md = f("md_in_w")[2 * D:], f("md_in_b")[2 * D:]
    a_dm = np.ascontiguousarray(wv_dm.T @ f("dm_out_w").T).astype(ml_dtypes.bfloat16)
    c_dm = bv_dm @ f("dm_out_w").T + f("dm_out_b")
    a_md = np.ascontiguousarray(wv_md.T @ f("md_out_w").T).astype(ml_dtypes.bfloat16)
    c_md = bv_md @ f("md_out_w").T + f("md_out_b")
    g_cat = np.concatenate([f("norm_d_g"), f("norm_m_g")])
    b_cat = np.concatenate([f("norm_d_b"), f("norm_m_b")])
    w1f = np.ascontiguousarray((f("ffn_w1") * g_cat[None, :]).T).astype(ml_dtypes.bfloat16)
    b1f = f("ffn_b1") + b_cat @ f("ffn_w1").T
    w2f = np.ascontiguousarray(f("ffn_w2").T).astype(ml_dtypes.bfloat16)
    b2 = f("ffn_b2")
    g_o, b_o = f("norm_out_g"), f("norm_out_b")

    flags = (bool(np.any(c_dm)), bool(np.any(c_md)), bool(np.any(b1f)),
             bool(np.any(b2)), bool(np.any(g_o != 1.0) or np.any(b_o)))

    key = (bc, NB, flags)
    if key not in _NC_CACHE:
        _NC_CACHE[key] = _build_nc(bc, flags)
    nc = _NC_CACHE[key]

    in_maps = []
    for c in range(N_CORES):
        sl = slice(c * bc, (c + 1) * bc)
        m = {
            "xd": np.ascontiguousarray(drug[sl].T).astype(ml_dtypes.bfloat16),
            "xm": np.ascontiguousarray(micro[sl].T).astype(ml_dtypes.bfloat16),
            "a_dm": a_dm, "a_md": a_md, "w1": w1f, "w2": w2f,
        }
        if flags[0]:
            m["c_dm"] = c_dm
        if flags[1]:
            m["c_md"] = c_md
        if flags[2]:
            m["b1"] = b1f
        if flags[3]:
            m["b2"] = b2
        if flags[4]:
            m["g_o"] = g_o
            m["b_o"] = b_o
        in_maps.append(m)

    res = run_bass_kernel_spmd(nc, in_maps, list(range(N_CORES)))
    LAST_RESULTS = res

    out = np.empty((b, D), np.float32)
    for c in range(N_CORES):
        out[c * bc:(c + 1) * bc] = res.results[c]["o"].T
    return out


# revision 10
# speedup vs baseline: 1.1550x; 1.0014x over previous
"""CrossAttentionFusion forward on 8 Trainium2 NeuronCores (pure data parallel).

Math folded on host (seq-len-1 MHA == two chained linears):
  d_att = micro @ A_dm + c_dm,  A_dm = Wv_dm.T @ Wout_dm.T
  m_att = drug  @ A_md + c_md
  u = drug + d_att ; w = micro + m_att
  xu = (u - mu)/sd ; xw likewise        (LN affine folded into W1)
  h1 = gelu([xu, xw] @ W1f + b1f),  W1f = (ffn_w1 * g_cat).T
  h2 = h1 @ W2f + b2,               W2f = ffn_w2.T
  out = ((h2 - mu)/sd) * g_out + b_out

Device layout: feature-major [feat(partition), batch(free)], batch sharded
across 8 cores, NB=256 batch columns per tile (32 tiles/core).

Software pipeline per iteration t (PE program order):
  attn(t) -> out-LN(t-2) -> FFN1(t-1) -> LN-stats(t) -> FFN2(t-1)
so the cross-engine LayerNorm stat chains (PE stats -> DVE smalls ->
GpSimd partition-broadcast -> DVE normalize) always have ~8us of
independent PE work in front of their consumers.  LN mean/rstd are
broadcast across partitions by GpSimd (partition_broadcast), not by PE
matmuls, freeing PE cycles and PSUM banks.
"""

import sys

if "/opt/trn_rl_repo" not in sys.path:
    sys.path.insert(0, "/opt/trn_rl_repo")

from contextlib import ExitStack

import ml_dtypes
import numpy as np

import concourse.bass as bass  # noqa: F401  (registers mybir lowering hooks)
import concourse.tile as tile
from concourse import bacc, mybir
from concourse.bass import ts, broadcast_tensor_aps
from concourse.bass_utils import run_bass_kernel_spmd

F32 = mybir.dt.float32
BF16 = mybir.dt.bfloat16
ACT = mybir.ActivationFunctionType

P = 128
D = 384
KD = D // P          # 3
DH = 2 * D           # 768
KH = DH // P         # 6
DF = 4 * D           # 1536
KF = DF // P         # 12
EPS = 1e-5
N_CORES = 8
B_FULL = 65536
BC = B_FULL // N_CORES   # 8192 rows per core
NB = 256                 # batch columns per on-chip tile
NT = BC // NB            # 32 tiles per core

_NC_CACHE = {}
LAST_RESULTS = None      # BassKernelResults of the most recent kernel() call


def _build_nc(bc, flags):
    use_c_dm, use_c_md, use_b1, use_b2, use_affine = flags
    nt = bc // NB
    nc = bacc.Bacc("TRN2", target_bir_lowering=False, debug=False,
                   num_devices=N_CORES)

    xd_d = nc.dram_tensor("xd", [D, bc], BF16, kind="ExternalInput")
    xm_d = nc.dram_tensor("xm", [D, bc], BF16, kind="ExternalInput")
    a_dm_d = nc.dram_tensor("a_dm", [D, D], BF16, kind="ExternalInput")
    a_md_d = nc.dram_tensor("a_md", [D, D], BF16, kind="ExternalInput")
    w1_d = nc.dram_tensor("w1", [DH, DF], BF16, kind="ExternalInput")
    w2_d = nc.dram_tensor("w2", [DF, D], BF16, kind="ExternalInput")
    c_dm_d = nc.dram_tensor("c_dm", [D], F32, kind="ExternalInput") if use_c_dm else None
    c_md_d = nc.dram_tensor("c_md", [D], F32, kind="ExternalInput") if use_c_md else None
    b1_d = nc.dram_tensor("b1", [DF], F32, kind="ExternalInput") if use_b1 else None
    b2_d = nc.dram_tensor("b2", [D], F32, kind="ExternalInput") if use_b2 else None
    g_o_d = nc.dram_tensor("g_o", [D], F32, kind="ExternalInput") if use_affine else None
    b_o_d = nc.dram_tensor("b_o", [D], F32, kind="ExternalInput") if use_affine else None
    o_d = nc.dram_tensor("o", [D, bc], F32, kind="ExternalOutput")

    xd_r = xd_d.ap().rearrange("(k p) n -> p k n", p=P)
    xm_r = xm_d.ap().rearrange("(k p) n -> p k n", p=P)
    o_r = o_d.ap().rearrange("(k p) n -> p k n", p=P)

    with tile.TileContext(nc) as tc, ExitStack() as ctx:
        wp = ctx.enter_context(tc.tile_pool(name="wts", bufs=1))
        xp = ctx.enter_context(tc.tile_pool(name="x", bufs=3))
        up = ctx.enter_context(tc.tile_pool(name="u", bufs=2))
        sqp = ctx.enter_context(tc.tile_pool(name="sq", bufs=2))
        xhp = ctx.enter_context(tc.tile_pool(name="xh", bufs=2))
        h1p = ctx.enter_context(tc.tile_pool(name="h1", bufs=2))
        h2p = ctx.enter_context(tc.tile_pool(name="h2", bufs=3))
        op_ = ctx.enter_context(tc.tile_pool(name="o", bufs=2))
        stp = ctx.enter_context(tc.tile_pool(name="st", bufs=4))
        bcp = ctx.enter_context(tc.tile_pool(name="bc", bufs=2))
        # PSUM: pbig 2 bufs x 2 banks + pf1 2 bufs x 1 bank + pst 2 banks = 8
        pbig = ctx.enter_context(tc.tile_pool(name="pbig", bufs=2, space="PSUM"))
        pf1 = ctx.enter_context(tc.tile_pool(name="pf1", bufs=2, space="PSUM"))
        pst = ctx.enter_context(tc.tile_pool(name="pst", bufs=1, space="PSUM"))

        a_dm_sb = wp.tile([P, KD, D], BF16)
        nc.gpsimd.dma_start(a_dm_sb[:], a_dm_d.ap().rearrange("(k p) m -> p k m", p=P))
        a_md_sb = wp.tile([P, KD, D], BF16)
        nc.gpsimd.dma_start(a_md_sb[:], a_md_d.ap().rearrange("(k p) m -> p k m", p=P))
        w1_sb = wp.tile([P, KH, DF], BF16)
        nc.gpsimd.dma_start(w1_sb[:], w1_d.ap().rearrange("(k p) m -> p k m", p=P))
        w2_sb = wp.tile([P, KF, D], BF16)
        nc.gpsimd.dma_start(w2_sb[:], w2_d.ap().rearrange("(k p) m -> p k m", p=P))

        ones_mean = wp.tile([P, 1], BF16)
        nc.vector.memset(ones_mean[:], 1.0 / D)
        ones16 = wp.tile([P, 1], BF16)
        nc.vector.memset(ones16[:], 1.0)
        eps_sb = wp.tile([1, 1], F32)
        nc.vector.memset(eps_sb[:], EPS)

        def vec_const(dram, nk, tag):
            t = wp.tile([P, nk], F32, tag=tag)
            nc.gpsimd.dma_start(t[:], dram.ap().rearrange("(k p) -> p k", p=P))
            return t

        c_dm_sb = vec_const(c_dm_d, KD, "c_dm") if use_c_dm else None
        c_md_sb = vec_const(c_md_d, KD, "c_md") if use_c_md else None
        b1_sb = vec_const(b1_d, KF, "b1") if use_b1 else None
        b2_sb = vec_const(b2_d, KD, "b2") if use_b2 else None
        g_o_sb = vec_const(g_o_d, KD, "g_o") if use_affine else None
        b_o_sb = vec_const(b_o_d, KD, "b_o") if use_affine else None

        # Persistent stats PSUM: slots 0..3 = {sum_u, ssq_u, sum_w, ssq_w}
        # (also reused as {sum_o, ssq_o} in slots 0..1 for the out-LN, which
        # runs at a disjoint point of the iteration).
        stat_ps = pst.tile([1, 4, NB], F32)

        def bsub(out_ap, a_ap, b_ap):
            a2, b2_ = broadcast_tensor_aps(a_ap, b_ap)
            o2, _ = broadcast_tensor_aps(out_ap, b_ap)
            nc.vector.tensor_sub(o2, a2, b2_)

        def bmul(out_ap, a_ap, b_ap):
            a2, b2_ = broadcast_tensor_aps(a_ap, b_ap)
            o2, _ = broadcast_tensor_aps(out_ap, b_ap)
            nc.vector.tensor_mul(o2, a2, b2_)

        def ln_smalls(sum_sl, ssq_sl, tag):
            """[1,NB] psum rows -> (mu16, inv16) bf16 SBUF on partition 0."""
            mu16 = stp.tile([1, NB], BF16, tag="mu" + tag)
            nc.vector.tensor_copy(mu16[:], sum_sl)
            mu2 = stp.tile([1, NB], F32, tag="mu2")
            nc.vector.tensor_mul(mu2[:], mu16[:], mu16[:])
            var = stp.tile([1, NB], F32, tag="var")
            nc.vector.scalar_tensor_tensor(var[:], ssq_sl, float(1.0 / D),
                                           mu2[:], mybir.AluOpType.mult,
                                           mybir.AluOpType.subtract)
            sd = stp.tile([1, NB], F32, tag="sd")
            nc.scalar.activation(sd[:], var[:], ACT.Sqrt, bias=eps_sb[:])
            inv = stp.tile([1, NB], F32, tag="inv")
            nc.vector.reciprocal_approx_fast(inv[:], sd[:])
            inv16 = stp.tile([1, NB], BF16, tag="inv" + tag)
            nc.vector.tensor_copy(inv16[:], inv[:])
            return mu16, inv16

        def bcast(vec16, tag):
            t = bcp.tile([P, 1, NB], BF16, tag=tag)
            nc.gpsimd.partition_broadcast(t[:, 0, :], vec16[:])
            return t

        state = {}

        def emit_attn(t, sl):
            xd = xp.tile([P, KD, NB], BF16, tag="xd")
            nc.sync.dma_start(xd[:], xd_r[:, :, sl])
            xm = xp.tile([P, KD, NB], BF16, tag="xm")
            nc.sync.dma_start(xm[:], xm_r[:, :, sl])

            def attn(a_sb, rhs, res, c_sb, tag):
                ps = pbig.tile([P, KD, NB], F32, tag="big")
                for m in range(KD):
                    for k in range(KD):
                        nc.tensor.matmul(ps[:, m, :],
                                         a_sb[:, k, ts(m, P)],
                                         rhs[:, k, :],
                                         start=(k == 0), stop=(k == KD - 1))
                v = up.tile([P, KD, NB], BF16, tag=tag)
                nc.vector.tensor_add(v[:], ps[:], res[:])
                if c_sb is not None:
                    for m in range(KD):
                        nc.vector.tensor_scalar_add(v[:, m, :], v[:, m, :],
                                                    c_sb[:, m:m + 1])
                return v

            u = attn(a_dm_sb, xm, xd, c_dm_sb, "u")
            w = attn(a_md_sb, xd, xm, c_md_sb, "w")
            state[t] = {"u": u, "w": w}

        def emit_uw_stats(t):
            st_ = state[t]
            u, w = st_["u"], st_["w"]
            sq_u = sqp.tile([P, KD, NB], BF16, tag="squ")
            nc.scalar.activation(sq_u[:], u[:], ACT.Square)
            sq_w = sqp.tile([P, KD, NB], BF16, tag="sqw")
            nc.scalar.activation(sq_w[:], w[:], ACT.Square)
            for k in range(KD):
                nc.tensor.matmul(stat_ps[0:1, 0, :], ones_mean[:], u[:, k, :],
                                 start=(k == 0), stop=(k == KD - 1))
            for k in range(KD):
                nc.tensor.matmul(stat_ps[0:1, 1, :], ones16[:], sq_u[:, k, :],
                                 start=(k == 0), stop=(k == KD - 1))
            for k in range(KD):
                nc.tensor.matmul(stat_ps[0:1, 2, :], ones_mean[:], w[:, k, :],
                                 start=(k == 0), stop=(k == KD - 1))
            for k in range(KD):
                nc.tensor.matmul(stat_ps[0:1, 3, :], ones16[:], sq_w[:, k, :],
                                 start=(k == 0), stop=(k == KD - 1))
            mu_u, inv_u = ln_smalls(stat_ps[0:1, 0, :], stat_ps[0:1, 1, :], "u")
            mu_w, inv_w = ln_smalls(stat_ps[0:1, 2, :], stat_ps[0:1, 3, :], "w")
            mu_bu = bcast(mu_u, "mbu")
            iv_bu = bcast(inv_u, "ibu")
            mu_bw = bcast(mu_w, "mbw")
            iv_bw = bcast(inv_w, "ibw")
            xh = xhp.tile([P, KH, NB], BF16, tag="xh")
            bsub(u[:], u[:], mu_bu[:])
            bmul(xh[:, 0:KD, :], u[:], iv_bu[:])
            bsub(w[:], w[:], mu_bw[:])
            bmul(xh[:, KD:KH, :], w[:], iv_bw[:])
            st_["xh"] = xh

        def emit_ffn1(t):
            st_ = state[t]
            xh = st_["xh"]
            h1 = h1p.tile([P, KF, NB], BF16, tag="h1")
            for g in range(KF // 2):
                ps = pf1.tile([P, 2, NB], F32, tag="f1")
                for mi in range(2):
                    m = g * 2 + mi
                    for k in range(KH):
                        nc.tensor.matmul(ps[:, mi, :],
                                         w1_sb[:, k, ts(m, P)],
                                         xh[:, k, :],
                                         start=(k == 0), stop=(k == KH - 1))
                if use_b1:
                    for mi in range(2):
                        m = g * 2 + mi
                        nc.scalar.activation(h1[:, m, :], ps[:, mi, :],
                                             ACT.Gelu, bias=b1_sb[:, m:m + 1])
                else:
                    nc.scalar.activation(h1[:, ts(g, 2), :], ps[:], ACT.Gelu)
            st_["h1"] = h1

        def emit_ffn2(t):
            st_ = state[t]
            h1 = st_["h1"]
            ps = pbig.tile([P, KD, NB], F32, tag="big")
            for m in range(KD):
                for k in range(KF):
                    nc.tensor.matmul(ps[:, m, :], w2_sb[:, k, ts(m, P)],
                                     h1[:, k, :],
                                     start=(k == 0), stop=(k == KF - 1))
            h2 = h2p.tile([P, KD, NB], BF16, tag="h2")
            if use_b2:
                for m in range(KD):
                    nc.scalar.activation(h2[:, m, :], ps[:, m, :],
                                         ACT.Identity, bias=b2_sb[:, m:m + 1])
            else:
                nc.scalar.activation(h2[:], ps[:], ACT.Copy)
            st_["h2"] = h2

        def emit_outln_stats(t):
            st_ = state[t]
            h2 = st_["h2"]
            sq_o = sqp.tile([P, KD, NB], BF16, tag="sqo")
            nc.scalar.activation(sq_o[:], h2[:], ACT.Square)
            # slots 0/1 of stat_ps are free here: previous iteration's u/w
            # smalls have already consumed them.
            for k in range(KD):
                nc.tensor.matmul(stat_ps[0:1, 0, :], ones_mean[:], h2[:, k, :],
                                 start=(k == 0), stop=(k == KD - 1))
            for k in range(KD):
                nc.tensor.matmul(stat_ps[0:1, 1, :], ones16[:], sq_o[:, k, :],
                                 start=(k == 0), stop=(k == KD - 1))

        def emit_outln_finish(t, sl):
            st_ = state.pop(t)
            h2 = st_["h2"]
            mu_o, inv_o = ln_smalls(stat_ps[0:1, 0, :], stat_ps[0:1, 1, :], "o")
            mu_bo = bcast(mu_o, "mbo")
            iv_bo = bcast(inv_o, "ibo")
            o = op_.tile([P, KD, NB], F32, tag="o")
            bsub(o[:], h2[:], mu_bo[:])
            bmul(o[:], o[:], iv_bo[:])
            if use_affine:
                for k in range(KD):
                    nc.vector.tensor_scalar(o[:, k, :], o[:, k, :],
                                            g_o_sb[:, k:k + 1],
                                            b_o_sb[:, k:k + 1],
                                            mybir.AluOpType.mult,
                                            mybir.AluOpType.add)
            nc.sync.dma_start(o_r[:, :, sl], o[:])

        for t in range(nt + 2):
            sl = slice(t * NB, (t + 1) * NB)
            if t < nt:
                emit_attn(t, sl)
            if t >= 2:
                emit_outln_stats(t - 2)
            if 1 <= t <= nt:
                emit_ffn1(t - 1)
            if t >= 2:
                osl = slice((t - 2) * NB, (t - 1) * NB)
                emit_outln_finish(t - 2, osl)
            if t < nt:
                emit_uw_stats(t)
            if 1 <= t <= nt:
                emit_ffn2(t - 1)

    nc.compile()
    return nc


def kernel(**inputs) -> np.ndarray:
    global LAST_RESULTS
    f = lambda k: np.asarray(inputs[k], np.float32)

    drug = f("drug_emb")
    micro = f("micro_emb")
    b = drug.shape[0]
    bc = b // N_CORES
    assert b % (N_CORES * NB) == 0

    # ---- host-side weight folding ----
    wv_dm, bv_dm = f("dm_in_w")[2 * D:], f("dm_in_b")[2 * D:]
    wv_md, bv_md = f("md_in_w")[2 * D:], f("md_in_b")[2 * D:]
    a_dm = np.ascontiguousarray(wv_dm.T @ f("dm_out_w").T).astype(ml_dtypes.bfloat16)
    c_dm = bv_dm @ f("dm_out_w").T + f("dm_out_b")
    a_md = np.ascontiguousarray(wv_md.T @ f("md_out_w").T).astype(ml_dtypes.bfloat16)
    c_md = bv_md @ f("md_out_w").T + f("md_out_b")
    g_cat = np.concatenate([f("norm_d_g"), f("norm_m_g")])
    b_cat = np.concatenate([f("norm_d_b"), f("norm_m_b")])
    w1f = np.ascontiguousarray((f("ffn_w1") * g_cat[None, :]).T).astype(ml_dtypes.bfloat16)
    b1f = f("ffn_b1") + b_cat @ f("ffn_w1").T
    w2f = np.ascontiguousarray(f("ffn_w2").T).astype(ml_dtypes.bfloat16)
    b2 = f("ffn_b2")
    g_o, b_o = f("norm_out_g"), f("norm_out_b")

    flags = (bool(np.any(c_dm)), bool(np.any(c_md)), bool(np.any(b1f)),
             bool(np.any(b2)), bool(np.any(g_o != 1.0) or np.any(b_o)))

    key = (bc, NB, flags)
    if key not in _NC_CACHE:
        _NC_CACHE[key] = _build_nc(bc, flags)
    nc = _NC_CACHE[key]

    in_maps = []
    for c in range(N_CORES):
        sl = slice(c * bc, (c + 1) * bc)
        m = {
            "xd": np.ascontiguousarray(drug[sl].T).astype(ml_dtypes.bfloat16),
            "xm": np.ascontiguousarray(micro[sl].T).astype(ml_dtypes.bfloat16),
            "a_dm": a_dm, "a_md": a_md, "w1": w1f, "w2": w2f,
        }
        if flags[0]:
            m["c_dm"] = c_dm
        if flags[1]:
            m["c_md"] = c_md
        if flags[2]:
            m["b1"] = b1f
        if flags[3]:
            m["b2"] = b2
        if flags[4]:
            m["g_o"] = g_o
            m["b_o"] = b_o
        in_maps.append(m)

    res = run_bass_kernel_spmd(nc, in_maps, list(range(N_CORES)))
    LAST_RESULTS = res

    out = np.empty((b, D), np.float32)
    for c in range(N_CORES):
        out[c * bc:(c + 1) * bc] = res.results[c]["o"].T
    return out
